# revision 12
# baseline (speedup 1.0000x reference)
"""Trainium2 Bass kernel for nn_CACBlock (multi-dilation depthwise conv +
top-k autocorrelation mixing + projection + residual LayerNorm).

Data-parallel over batch: 8 cores x 4 batches each. Full inputs in,
full outputs out; all sharding/gather inside kernel().
"""

import numpy as np
import ml_dtypes

import concourse.bass as bass
import concourse.bacc as bacc
import concourse.tile as tile
import concourse.mybir as mybir
from concourse.bass_utils import run_bass_kernel_spmd

F32 = mybir.dt.float32
BF16 = mybir.dt.bfloat16
I32 = mybir.dt.int32
U32 = mybir.dt.uint32

B, L, D = 32, 2048, 512
N_CORES = 8
B_LOC = B // N_CORES
D2 = 2 * D
TOPK = 5
MAXC = 32
LN_EPS = 1e-5
P = 128
N_DBLK = D // P          # 4 d-blocks per branch
N_KBLK = D2 // P         # 8 contraction blocks
N_LT = L // P            # 16 l-tiles per batch


def _cand_lags() -> np.ndarray:
    max_lag = min(L - 1, 168)
    num = min(max_lag, MAXC)
    return np.linspace(1, max_lag, num).astype(np.int64)


def _tap_table(conv_w0, conv_w1, conv_w2):
    """Combined depthwise kernel: offset -> per-channel weight [D]."""
    taps = {}
    for w, dil in ((conv_w0, 1), (conv_w1, 2), (conv_w2, 4)):
        k = w.shape[-1]
        for j in range(k):
            off = (j - k // 2) * dil
            taps[off] = taps.get(off, 0.0) + w[:, 0, j].astype(np.float64)
    offs = sorted(taps)
    tab = np.stack([taps[o] for o in offs], axis=1)  # [D, n_taps]
    return offs, tab.astype(np.float32)


def _build(n_taps: int, need_pb: bool, need_gb: bool):
    """Build + compile the per-core program (identical across cores)."""
    nc = bacc.Bacc(
        "TRN2",
        target_bir_lowering=False,
        debug=False,
        num_devices=N_CORES,
    )
    x_d = nc.dram_tensor("x", [B_LOC, L, D], F32, kind="ExternalInput")
    xbf_d = nc.dram_tensor("xbf", [B_LOC, L, D], BF16, kind="ExternalInput")
    pwt_d = nc.dram_tensor("pwt", [D2, D], BF16, kind="ExternalInput")
    ktap_d = nc.dram_tensor("ktap", [D, n_taps], F32, kind="ExternalInput")
    tst_d = nc.dram_tensor("tstart", [1, MAXC], I32, kind="ExternalInput")
    icnt_d = nc.dram_tensor("invcnt", [MAXC, 1], F32, kind="ExternalInput")
    if need_pb:
        pb_d = nc.dram_tensor("pb", [1, D], F32, kind="ExternalInput")
    if need_gb:
        g_d = nc.dram_tensor("lng", [1, D], F32, kind="ExternalInput")
        bb_d = nc.dram_tensor("lnb", [1, D], F32, kind="ExternalInput")
    out_d = nc.dram_tensor("out", [B_LOC, L, D], F32, kind="ExternalOutput")

    lags = _cand_lags()
    Add = mybir.AluOpType.add
    Sub = mybir.AluOpType.subtract
    Mult = mybir.AluOpType.mult
    Copy = mybir.ActivationFunctionType.Copy
    Square = mybir.ActivationFunctionType.Square
    Sqrt = mybir.ActivationFunctionType.Sqrt

    with tile.TileContext(nc) as tc:
        with (
            tc.tile_pool(name="const", bufs=1) as cpool,
            tc.tile_pool(name="xf", bufs=20) as xfpool,
            tc.tile_pool(name="xt2", bufs=6) as xtpool,
            tc.tile_pool(name="acc", bufs=5) as accpool,
            tc.tile_pool(name="sc", bufs=1) as scpool,
            tc.tile_pool(name="xmrp", bufs=2) as xmrpool,
            tc.tile_pool(name="tiny", bufs=4) as tpool,
            tc.tile_pool(name="junk", bufs=3) as jpool,
            tc.tile_pool(name="outp", bufs=4) as opool,
            tc.tile_pool(name="psum", bufs=4, space=bass.MemorySpace.PSUM) as pspool,
        ):
            # ---- constants (loaded once) ----
            pwt_sb = cpool.tile([P, N_KBLK, D], BF16)  # [p, k, o]
            nc.sync.dma_start(
                pwt_sb[:], pwt_d[:, :].rearrange("(k p) o -> p k o", p=P)
            )
            ktap_sb = cpool.tile([P, N_DBLK, n_taps], F32)
            nc.sync.dma_start(
                ktap_sb[:], ktap_d[:, :].rearrange("(q p) t -> p q t", p=P)
            )
            tst_sb = cpool.tile([1, MAXC], I32)
            nc.sync.dma_start(tst_sb[:], tst_d[:, :])
            eps_sb = cpool.tile([P, 1], F32)
            nc.vector.memset(eps_sb[:], LN_EPS)
            icnt_sb = cpool.tile([MAXC, 1], F32)
            nc.sync.dma_start(icnt_sb[:], icnt_d[:, :])
            if need_pb:
                pb_r = cpool.tile([1, D], F32)
                nc.sync.dma_start(pb_r[:], pb_d[:, :])
                pb_sb = cpool.tile([P, D], F32)
                nc.gpsimd.partition_broadcast(pb_sb[:], pb_r[0:1, :])
            if need_gb:
                g_r = cpool.tile([1, D], F32)
                nc.sync.dma_start(g_r[:], g_d[:, :])
                g_sb = cpool.tile([P, D], F32)
                nc.gpsimd.partition_broadcast(g_sb[:], g_r[0:1, :])
                bb_r = cpool.tile([1, D], F32)
                nc.sync.dma_start(bb_r[:], bb_d[:, :])
                bb_sb = cpool.tile([P, D], F32)
                nc.gpsimd.partition_broadcast(bb_sb[:], bb_r[0:1, :])

            for b in range(B_LOC):
                # ---- load x (f32) as 16 l-tiles [128, 512] ----
                xf = []
                for t in range(N_LT):
                    xt = xfpool.tile([P, D], F32, tag="xf")
                    nc.sync.dma_start(xt[:], x_d[b, t * P:(t + 1) * P, :])
                    xf.append(xt)

                # ---- transposed bf16 copy: xT2[q] = [128 d, 4096 l] ----
                xt2 = []
                for q in range(N_DBLK):
                    xq = xtpool.tile([P, 2 * L], BF16, tag="xt2")
                    nc.sync.dma_start(
                        xq[:, 0:L],
                        xbf_d[b, :, q * P:(q + 1) * P],
                        transpose=True,
                    )
                    nc.sync.dma_start(xq[:, L:2 * L], xq[:, 0:L])
                    xt2.append(xq)

                # ---- xm (f32): ACT copy-accumulate row sums ----
                xmc = tpool.tile([P, N_LT], F32, tag="xmc")
                for t in range(N_LT):
                    jt = jpool.tile([P, D], BF16, tag="jact")
                    nc.scalar.activation(
                        jt[:], xf[t][:], Copy, accum_out=xmc[:, t:t + 1]
                    )
                # transpose-gather into a single row [1, 2048] (sum*512).
                # out free dim reordered (p,t)-major to match the in AP's
                # partition-major iteration: l = t*128 + p.
                xmr = xmrpool.tile([1, L], F32, tag="xmr")
                for t in range(N_LT):
                    nc.sync.dma_start(
                        xmr[0:1, t * P:(t + 1) * P], xmc[:, t:t + 1]
                    )

                # ---- autocorrelation scores (f32, exact) ----
                shift = scpool.tile([MAXC, L], F32, tag="shift")
                nc.vector.memset(shift[:], 0.0)
                for i, lag in enumerate(lags):
                    lag = int(lag)
                    nc.sync.dma_start(
                        shift[i:i + 1, 0:L - lag], xmr[0:1, lag:L]
                    )
                bcast = scpool.tile([MAXC, L], F32, tag="bcast")
                nc.gpsimd.partition_broadcast(bcast[:], xmr[0:1, :])
                sraw = tpool.tile([MAXC, 1], F32, tag="sraw")
                jsc = scpool.tile([MAXC, L], F32, tag="jsc")
                nc.vector.scalar_tensor_tensor(
                    jsc[:], shift[:], 1.0, bcast[:], Mult, Mult,
                    accum_out=sraw[:],
                )
                scores = tpool.tile([MAXC, 1], F32, tag="scores")
                nc.vector.tensor_scalar(
                    scores[:], sraw[:], icnt_sb[:], None, Mult
                )
                # row layout [1, 32] for top-k (partition-major on both sides)
                srow = tpool.tile([1, MAXC], F32, tag="srow")
                nc.sync.dma_start(srow[0:1, :], scores[:, :])

                # ---- top-5 ----
                vmax = tpool.tile([1, 8], F32, tag="vmax")
                idx8 = tpool.tile([1, 8], U32, tag="idx8")
                nc.vector.max_with_indices(vmax[:], idx8[:], srow[:])
                dsum = tpool.tile([1, 1], F32, tag="dsum")
                nc.vector.tensor_reduce(
                    dsum[:], vmax[0:1, 0:TOPK], mybir.AxisListType.X, Add
                )
                nc.vector.tensor_scalar(dsum[:], dsum[:], 1e-6, None, Add)
                dinv = tpool.tile([1, 1], F32, tag="dinv")
                nc.vector.reciprocal(dinv[:], dsum[:])
                w5 = tpool.tile([1, TOPK], F32, tag="w5")
                nc.vector.tensor_scalar(
                    w5[:], vmax[0:1, 0:TOPK], dinv[:], None, Mult
                )
                # broadcast weights to [128,1] bf16 columns
                wbc = tpool.tile([P, TOPK], F32, tag="wbc")
                nc.gpsimd.partition_broadcast(wbc[:], w5[0:1, :])
                # dynamic start offsets: start = 2048 - lags[idx_i]
                starts = []
                for i in range(TOPK):
                    r1 = nc.alloc_register(mybir.EngineType.DVE, f"ix{b}_{i}")
                    nc.vector.reg_load(r1, idx8[0:1, i:i + 1])
                    s1 = nc.snap(r1, donate=True, min_val=0, max_val=MAXC - 1)
                    r2 = nc.alloc_register(mybir.EngineType.DVE, f"st{b}_{i}")
                    nc.vector.reg_load(r2, tst_sb[0:1, bass.ds(s1, 1)])
                    s2 = nc.snap(
                        r2, donate=True, min_val=L - int(lags[-1]), max_val=L - 1
                    )
                    starts.append(s2)

                # ---- conv branch: 11 static taps on [d, l] bf16 ----
                conv = []
                for q in range(N_DBLK):
                    cq = accpool.tile([P, L], BF16, tag="conv")
                    for j, off in enumerate(tap_offsets):
                        ks = ktap_sb[:, q, j:j + 1]
                        lo_o, hi_o = max(0, -off), L - max(0, off)
                        lo_i = lo_o + off
                        src = xt2[q][:, lo_i:lo_i + (hi_o - lo_o)]
                        if j == 0:
                            assert off == 0  # full-range init tap
                            nc.vector.tensor_scalar(
                                cq[:, lo_o:hi_o], src, ks, None, Mult
                            )
                        else:
                            nc.vector.scalar_tensor_tensor(
                                cq[:, lo_o:hi_o], src, ks, cq[:, lo_o:hi_o],
                                Mult, Add,
                            )
                    conv.append(cq)

                # ---- auto branch: 5 dynamic circular shifts ----
                auto = []
                for q in range(N_DBLK):
                    aq = accpool.tile([P, L], BF16, tag="auto")
                    for i in range(TOPK):
                        src = xt2[q][:, bass.ds(starts[i], L)]
                        if i == 0:
                            nc.vector.tensor_scalar(
                                aq[:], src, wbc[:, 0:1], None, Mult
                            )
                        else:
                            nc.vector.scalar_tensor_tensor(
                                aq[:], src, wbc[:, i:i + 1], aq[:], Mult, Add
                            )
                    auto.append(aq)

                # ---- projection + residual + LN ----
                hs = tpool.tile([P, N_LT], F32, tag="hs")
                hs2 = tpool.tile([P, N_LT], F32, tag="hs2")
                for t in range(N_LT):
                    ps = pspool.tile([P, D], F32, tag="ps")
                    for k in range(N_KBLK):
                        lhs = (conv[k] if k < N_DBLK else auto[k - N_DBLK])
                        nc.tensor.matmul(
                            ps[:],
                            lhs[:, t * P:(t + 1) * P],
                            pwt_sb[:, k, :],
                            start=(k == 0),
                            stop=(k == N_KBLK - 1),
                        )
                    if need_pb:
                        nc.vector.tensor_tensor(ps[:], ps[:], pb_sb[:], Add)
                    # h = psum + x (in place over x tile), row-sums for free
                    nc.vector.scalar_tensor_tensor(
                        xf[t][:], ps[:], 1.0, xf[t][:], Mult, Add,
                        accum_out=hs[:, t:t + 1],
                    )
                    jq = jpool.tile([P, D], BF16, tag="jsq")
                    nc.scalar.activation(
                        jq[:], xf[t][:], Square, accum_out=hs2[:, t:t + 1]
                    )

                # batched LN stats [128, 16]
                mu = tpool.tile([P, N_LT], F32, tag="mu")
                nc.vector.tensor_scalar(mu[:], hs[:], 1.0 / D, None, Mult)
                var = tpool.tile([P, N_LT], F32, tag="var")
                nc.vector.tensor_scalar(var[:], hs2[:], 1.0 / D, None, Mult)
                musq = tpool.tile([P, N_LT], F32, tag="musq")
                nc.vector.tensor_tensor(musq[:], mu[:], mu[:], Mult)
                nc.vector.tensor_tensor(var[:], var[:], musq[:], Sub)
                sd = tpool.tile([P, N_LT], F32, tag="sd")
                nc.scalar.activation(sd[:], var[:], Sqrt, bias=eps_sb[:])
                rstd = tpool.tile([P, N_LT], F32, tag="rstd")
                nc.vector.reciprocal(rstd[:], sd[:])

                for t in range(N_LT):
                    ot = opool.tile([P, D], F32, tag="out")
                    nc.gpsimd.tensor_scalar(
                        ot[:], xf[t][:], mu[:, t:t + 1], rstd[:, t:t + 1],
                        Sub, Mult,
                    )
                    if need_gb:
                        nc.vector.tensor_tensor(ot[:], ot[:], g_sb[:], Mult)
                        nc.vector.tensor_tensor(ot[:], ot[:], bb_sb[:], Add)
                    nc.sync.dma_start(out_d[b, t * P:(t + 1) * P, :], ot[:])

    nc.compile()
    return nc


_CACHE: dict = {}
tap_offsets: list = []


def _prepare(x, conv_w0, conv_w1, conv_w2, proj_w, proj_b, ln_g, ln_b):
    global tap_offsets
    x = np.asarray(x, dtype=np.float32)
    conv_w0 = np.asarray(conv_w0, np.float32)
    conv_w1 = np.asarray(conv_w1, np.float32)
    conv_w2 = np.asarray(conv_w2, np.float32)
    proj_w = np.asarray(proj_w, np.float32)
    proj_b = np.asarray(proj_b, np.float32)
    ln_g = np.asarray(ln_g, np.float32)
    ln_b = np.asarray(ln_b, np.float32)

    offs, tab = _tap_table(conv_w0, conv_w1, conv_w2)
    # order taps so the full-range center tap (offset 0) initializes acc
    order = sorted(range(len(offs)), key=lambda j: (offs[j] != 0, j))
    tap_offsets = [offs[j] for j in order]
    tab = tab[:, order]

    need_pb = bool(np.any(proj_b != 0.0))
    need_gb = bool(np.any(ln_g != 1.0) or np.any(ln_b != 0.0))
    key = (len(tap_offsets), need_pb, need_gb)

    lags = _cand_lags()
    tstart = (L - lags.astype(np.int64)).astype(np.int32).reshape(1, MAXC)
    invcnt = (1.0 / ((L - lags.astype(np.float64)) * D * D)).astype(
        np.float32).reshape(MAXC, 1)

    xbf = x.astype(ml_dtypes.bfloat16)
    pwt = np.ascontiguousarray(proj_w.T).astype(ml_dtypes.bfloat16)
    ktap = tab.astype(np.float32)

    in_maps = []
    for c in range(N_CORES):
        m = {
            "x": np.ascontiguousarray(x[c * B_LOC:(c + 1) * B_LOC]),
            "xbf": np.ascontiguousarray(xbf[c * B_LOC:(c + 1) * B_LOC]),
            "pwt": pwt,
            "ktap": ktap,
            "tstart": tstart,
            "invcnt": invcnt,
        }
        if need_pb:
            m["pb"] = proj_b.reshape(1, D)
        if need_gb:
            m["lng"] = ln_g.reshape(1, D)
            m["lnb"] = ln_b.reshape(1, D)
        in_maps.append(m)
    return key, in_maps


def kernel(x, conv_w0, conv_w1, conv_w2, proj_w, proj_b, ln_g, ln_b):
    key, in_maps = _prepare(
        x, conv_w0, conv_w1, conv_w2, proj_w, proj_b, ln_g, ln_b
    )
    if key not in _CACHE:
        _CACHE[key] = _build(*key)
    nc = _CACHE[key]
    res = run_bass_kernel_spmd(nc, in_maps, list(range(N_CORES)))
    out = np.concatenate([res.results[c]["out"] for c in range(N_CORES)], axis=0)
    return out.astype(np.float32)


# revision 25
# speedup vs baseline: 11.4482x; 11.4482x over previous
"""Trainium2 Bass kernel for nn_CACBlock (multi-dilation depthwise conv +
top-k autocorrelation mixing + projection + residual LayerNorm).

Data-parallel over batch: 8 cores x 4 batches each. Full inputs in,
full outputs out; all sharding/gather inside kernel().
"""

import numpy as np
import ml_dtypes

import concourse.bass as bass
import concourse.bacc as bacc
import concourse.tile as tile
import concourse.mybir as mybir
from concourse.bass_utils import run_bass_kernel_spmd

F32 = mybir.dt.float32
BF16 = mybir.dt.bfloat16
I32 = mybir.dt.int32
U32 = mybir.dt.uint32

B, L, D = 32, 2048, 512
N_CORES = 8
B_LOC = B // N_CORES
D2 = 2 * D
TOPK = 5
MAXC = 32
LN_EPS = 1e-5
P = 128
N_DBLK = D // P          # 4 d-blocks per branch
N_KBLK = D2 // P         # 8 contraction blocks
N_LT = L // P            # 16 l-tiles per batch
MAX_N = 512              # PE moving free dim
XT2W = L + MAX_N         # x + wrap margin for rotations


def _cand_lags() -> np.ndarray:
    max_lag = min(L - 1, 168)
    num = min(max_lag, MAXC)
    return np.linspace(1, max_lag, num).astype(np.int64)


def _tap_table(conv_w0, conv_w1, conv_w2):
    """Combined depthwise kernel: offset -> per-channel weight [D]."""
    taps = {}
    for w, dil in ((conv_w0, 1), (conv_w1, 2), (conv_w2, 4)):
        k = w.shape[-1]
        for j in range(k):
            off = (j - k // 2) * dil
            taps[off] = taps.get(off, 0.0) + w[:, 0, j].astype(np.float64)
    offs = sorted(taps)
    tab = np.stack([taps[o] for o in offs], axis=1)  # [D, n_taps]
    return offs, tab.astype(np.float32)


def _build(n_taps: int, need_pb: bool, need_gb: bool):
    """Build + compile the per-core program (identical across cores)."""
    nc = bacc.Bacc(
        "TRN2",
        target_bir_lowering=False,
        debug=False,
        num_devices=N_CORES,
    )
    x_d = nc.dram_tensor("x", [B_LOC, L, D], F32, kind="ExternalInput")
    xbf_d = nc.dram_tensor("xbf", [B_LOC, L, D], BF16, kind="ExternalInput")
    pwt_d = nc.dram_tensor("pwt", [D2, D], BF16, kind="ExternalInput")
    ktap_d = nc.dram_tensor("ktap", [D, n_taps], F32, kind="ExternalInput")
    tst_d = nc.dram_tensor("tstart", [1, MAXC], I32, kind="ExternalInput")
    icnt_d = nc.dram_tensor("invcnt", [MAXC, 1], F32, kind="ExternalInput")
    idt_d = nc.dram_tensor("idt", [P, P], BF16, kind="ExternalInput")
    if need_pb:
        pb_d = nc.dram_tensor("pb", [1, D], F32, kind="ExternalInput")
    if need_gb:
        g_d = nc.dram_tensor("lng", [1, D], F32, kind="ExternalInput")
        bb_d = nc.dram_tensor("lnb", [1, D], F32, kind="ExternalInput")
    out_d = nc.dram_tensor("out", [B_LOC, L, D], F32, kind="ExternalOutput")

    lags = _cand_lags()
    Add = mybir.AluOpType.add
    Sub = mybir.AluOpType.subtract
    Mult = mybir.AluOpType.mult
    Copy = mybir.ActivationFunctionType.Copy
    Square = mybir.ActivationFunctionType.Square
    Sqrt = mybir.ActivationFunctionType.Sqrt

    with tile.TileContext(nc) as tc:
        with (
            tc.tile_pool(name="const", bufs=1) as cpool,
            tc.tile_pool(name="xf", bufs=8) as xfpool,
            tc.tile_pool(name="xr", bufs=6) as xrpool,
            tc.tile_pool(name="xt2", bufs=8) as xtpool,
            tc.tile_pool(name="acc", bufs=8) as accpool,
            tc.tile_pool(name="aacc", bufs=5) as aaccpool,
            tc.tile_pool(name="sc", bufs=1) as scpool,
            tc.tile_pool(name="xmrp", bufs=1) as xmrpool,
            tc.tile_pool(name="tiny", bufs=4) as tpool,
            tc.tile_pool(name="junk", bufs=3) as jpool,
            tc.tile_pool(name="outp", bufs=3) as opool,
            tc.tile_pool(name="hp", bufs=18) as hpool,
            tc.tile_pool(name="dg", bufs=2) as dgpool,
            tc.tile_pool(name="psum", bufs=6, space=bass.MemorySpace.PSUM) as pspool,
            tc.tile_pool(name="apsum", bufs=2, space=bass.MemorySpace.PSUM) as apspool,
        ):
            # ---- constants (loaded once) ----
            pwt_sb = cpool.tile([P, N_KBLK, D], BF16)  # [p, k, o]
            nc.sync.dma_start(
                pwt_sb[:], pwt_d[:, :].rearrange("(k p) o -> p k o", p=P)
            )
            ktap_sb = cpool.tile([P, N_DBLK, n_taps], F32)
            nc.sync.dma_start(
                ktap_sb[:], ktap_d[:, :].rearrange("(q p) t -> p q t", p=P)
            )
            tst_sb = cpool.tile([1, MAXC], I32)
            nc.sync.dma_start(tst_sb[:], tst_d[:, :])
            eps_sb = cpool.tile([P, 1], F32)
            nc.vector.memset(eps_sb[:], LN_EPS)
            idt_sb = cpool.tile([P, P], BF16)
            nc.sync.dma_start(idt_sb[:], idt_d[:, :])
            icnt_sb = cpool.tile([MAXC, 1], F32)
            nc.sync.dma_start(icnt_sb[:], icnt_d[:, :])
            if need_pb:
                pb_r = cpool.tile([1, D], F32)
                nc.sync.dma_start(pb_r[:], pb_d[:, :])
                pb_sb = cpool.tile([P, D], F32)
                nc.gpsimd.partition_broadcast(pb_sb[:], pb_r[0:1, :])
            if need_gb:
                g_r = cpool.tile([1, D], F32)
                nc.sync.dma_start(g_r[:], g_d[:, :])
                g_sb = cpool.tile([P, D], F32)
                nc.gpsimd.partition_broadcast(g_sb[:], g_r[0:1, :])
                bb_r = cpool.tile([1, D], F32)
                nc.sync.dma_start(bb_r[:], bb_d[:, :])
                bb_sb = cpool.tile([P, D], F32)
                nc.gpsimd.partition_broadcast(bb_sb[:], bb_r[0:1, :])

            def emit_A(b):
                st = {}

                # ---- transposed bf16 copy: xT2[q] = [128 d, 4096 l] ----
                xt2 = []
                for q in range(N_DBLK):
                    xq = xtpool.tile([P, XT2W], BF16, tag="xt2")
                    nc.sync.dma_start(
                        xq[:, 0:L],
                        xbf_d[b, :, q * P:(q + 1) * P],
                        transpose=True,
                    )
                    nc.sync.dma_start(xq[:, L:XT2W], xq[:, 0:MAX_N])
                    xt2.append(xq)

                # ---- conv branch: chains q0-2 on DVE, q3 on Pool ----
                conv = []
                for q in range(N_DBLK):

                    cq = accpool.tile([P, L], BF16, tag="conv")
                    for j, off in enumerate(tap_offsets):
                        eng = nc.vector
                        ks = ktap_sb[:, q, j:j + 1]
                        lo_o, hi_o = max(0, -off), L - max(0, off)
                        lo_i = lo_o + off
                        src = xt2[q][:, lo_i:lo_i + (hi_o - lo_o)]
                        if j == 0:
                            assert off == 0  # full-range init tap
                            eng.tensor_scalar(
                                cq[:, lo_o:hi_o], src, ks, None, Mult
                            )
                        else:
                            eng.scalar_tensor_tensor(
                                cq[:, lo_o:hi_o], src, ks, cq[:, lo_o:hi_o],
                                Mult, Add,
                            )
                    conv.append(cq)
                st["conv"] = conv

                # ---- xm (f32): ACT copy-accumulate row sums ----
                xmc = tpool.tile([P, N_LT], F32, tag="xmc")
                for t in range(N_LT):
                    xt = xfpool.tile([P, D], F32, tag="xf")
                    nc.sync.dma_start(xt[:], x_d[b, t * P:(t + 1) * P, :])
                    jt = jpool.tile([P, D], BF16, tag="jact")
                    nc.scalar.activation(
                        jt[:], xt[:], Copy, accum_out=xmc[:, t:t + 1]
                    )
                xmr = xmrpool.tile([1, L], F32, tag="xmr")
                for t in range(N_LT):
                    eng = nc.sync if t % 2 == 0 else nc.scalar
                    eng.dma_start(
                        xmr[0:1, t * P:(t + 1) * P], xmc[:, t:t + 1]
                    )

                # ---- autocorrelation scores (f32, exact) ----
                shift = scpool.tile([MAXC, L], F32, tag="shift")
                nc.gpsimd.memset(shift[:], 0.0)
                for i, lag in enumerate(lags):
                    lag = int(lag)
                    eng = nc.sync if i % 2 == 0 else nc.scalar
                    eng.dma_start(
                        shift[i:i + 1, 0:L - lag], xmr[0:1, lag:L]
                    )
                bcast = scpool.tile([MAXC, L], F32, tag="bcast")
                nc.gpsimd.partition_broadcast(bcast[:], xmr[0:1, :])
                sraw = tpool.tile([MAXC, 1], F32, tag="sraw")
                nc.vector.scalar_tensor_tensor(
                    shift[:], shift[:], 1.0, bcast[:], Mult, Mult,
                    accum_out=sraw[:],
                )
                scores = tpool.tile([MAXC, 1], F32, tag="scores")
                nc.vector.tensor_scalar(
                    scores[:], sraw[:], icnt_sb[:], None, Mult
                )
                srow = tpool.tile([1, MAXC], F32, tag="srow")
                nc.sync.dma_start(srow[0:1, :], scores[:, :])

                # ---- top-5 ----
                vmax = tpool.tile([1, 8], F32, tag="vmax")
                idx8 = tpool.tile([1, 8], U32, tag="idx8")
                nc.vector.max_with_indices(vmax[:], idx8[:], srow[:])
                dsum = tpool.tile([1, 1], F32, tag="dsum")
                nc.vector.tensor_reduce(
                    dsum[:], vmax[0:1, 0:TOPK], mybir.AxisListType.X, Add
                )
                nc.vector.tensor_scalar(dsum[:], dsum[:], 1e-6, None, Add)
                dinv = tpool.tile([1, 1], F32, tag="dinv")
                nc.vector.reciprocal(dinv[:], dsum[:])
                w5 = tpool.tile([1, TOPK], F32, tag="w5")
                nc.vector.tensor_scalar(
                    w5[:], vmax[0:1, 0:TOPK], dinv[:], None, Mult
                )
                wbc = tpool.tile([P, TOPK], F32, tag="wbc")
                nc.gpsimd.partition_broadcast(wbc[:], w5[0:1, :])
                # diag(w_i) lhsT tiles for the PE auto path
                diags = []
                for i in range(TOPK):
                    dg = dgpool.tile([P, P], BF16, tag=f"diag{i}")
                    nc.vector.tensor_scalar(
                        dg[:], idt_sb[:], wbc[:, i:i + 1], None, Mult
                    )
                    diags.append(dg)
                st["diags"] = diags
                st["idx8"] = idx8
                st["xt2"] = xt2
                return st

            def emit_A2(b, st):
                # auto branch on PE: psum += diag(w_i) @ x[:, rot_i] per chunk
                xt2, diags, idx8 = st["xt2"], st["diags"], st["idx8"]
                NCH = L // MAX_N
                starts = []
                for i in range(TOPK):
                    r1 = nc.alloc_register(mybir.EngineType.PE, f"ix{b}_{i}")
                    nc.tensor.reg_load(r1, idx8[0:1, i:i + 1])
                    s1 = nc.snap(r1, donate=True, min_val=0, max_val=MAXC - 1)
                    r2 = nc.alloc_register(mybir.EngineType.PE, f"st{b}_{i}")
                    nc.tensor.reg_load(r2, tst_sb[0:1, bass.ds(s1, 1)])
                    row = []
                    for c in range(NCH):
                        r3 = nc.alloc_register(mybir.EngineType.PE, f"sc{b}_{i}_{c}")
                        nc.tensor.reg_alu(r3, r2, c * MAX_N - L, Add)
                        if c == 0:
                            nc.tensor.reg_alu(r3, r3, L, Add)
                            lo = L - int(lags[-1])
                        else:
                            lo = c * MAX_N - int(lags[-1])
                        s3 = nc.snap(
                            r3, donate=True, min_val=lo,
                            max_val=lo + int(lags[-1]) - 1,
                        )
                        row.append(s3)
                    starts.append(row)
                auto = []
                for q in range(N_DBLK):
                    aq = aaccpool.tile([P, L], BF16, tag="auto")
                    for c in range(NCH):
                        ap = apspool.tile([P, MAX_N], F32, tag="aps")
                        for i in range(TOPK):
                            nc.tensor.matmul(
                                ap[:],
                                diags[i][:],
                                xt2[q][:, bass.ds(starts[i][c], MAX_N)],
                                start=(i == 0),
                                stop=(i == TOPK - 1),
                            )
                        nc.scalar.activation(
                            aq[:, c * MAX_N:(c + 1) * MAX_N], ap[:], Copy
                        )
                    auto.append(aq)
                st["auto"] = auto
                return st

            def emit_B(b, st):
                conv, auto = st["conv"], st["auto"]
                xr = []
                for t in range(N_LT):
                    xt = xrpool.tile([P, D], F32, tag="xr")
                    nc.scalar.dma_start(xt[:], x_d[b, t * P:(t + 1) * P, :])
                    xr.append(xt)
                hs = tpool.tile([P, N_LT], F32, tag="hs")
                hs2 = tpool.tile([P, N_LT], F32, tag="hs2")
                hts = []
                for t in range(N_LT):
                    ps = pspool.tile([P, D], F32, tag="ps")
                    for k in range(N_KBLK):
                        lhs = (conv[k] if k < N_DBLK else auto[k - N_DBLK])
                        nc.tensor.matmul(
                            ps[:],
                            lhs[:, t * P:(t + 1) * P],
                            pwt_sb[:, k, :],
                            start=(k == 0),
                            stop=(k == N_KBLK - 1),
                        )
                    if need_pb:
                        nc.vector.tensor_tensor(ps[:], ps[:], pb_sb[:], Add)
                    ht = hpool.tile([P, D], BF16, tag="h")
                    nc.vector.scalar_tensor_tensor(
                        ht[:], ps[:], 1.0, xr[t][:], Mult, Add,
                        accum_out=hs[:, t:t + 1],
                    )
                    hts.append(ht)
                    jq = jpool.tile([P, D], BF16, tag="jact")
                    nc.scalar.activation(
                        jq[:], ht[:], Square, accum_out=hs2[:, t:t + 1]
                    )

                mu = tpool.tile([P, N_LT], F32, tag="mu")
                nc.vector.tensor_scalar(mu[:], hs[:], 1.0 / D, None, Mult)
                var = tpool.tile([P, N_LT], F32, tag="var")
                nc.vector.tensor_scalar(var[:], hs2[:], 1.0 / D, None, Mult)
                musq = tpool.tile([P, N_LT], F32, tag="musq")
                nc.vector.tensor_tensor(musq[:], mu[:], mu[:], Mult)
                nc.vector.tensor_tensor(var[:], var[:], musq[:], Sub)
                sd = tpool.tile([P, N_LT], F32, tag="sd")
                nc.scalar.activation(sd[:], var[:], Sqrt, bias=eps_sb[:])
                rstd = tpool.tile([P, N_LT], F32, tag="rstd")
                nc.vector.reciprocal(rstd[:], sd[:])
                for t in range(N_LT):
                    ot = opool.tile([P, D], F32, tag="out")
                    nc.gpsimd.tensor_scalar(
                        ot[:], hts[t][:], mu[:, t:t + 1], rstd[:, t:t + 1],
                        Sub, Mult,
                    )
                    if need_gb:
                        nc.vector.tensor_tensor(ot[:], ot[:], g_sb[:], Mult)
                        nc.vector.tensor_tensor(ot[:], ot[:], bb_sb[:], Add)
                    nc.gpsimd.dma_start(out_d[b, t * P:(t + 1) * P, :], ot[:])

            # software pipeline: A1(0) A2(0) A1(1) B(0) A2(1) A1(2) ...
            sts = {0: emit_A(0)}
            emit_A2(0, sts[0])
            for b in range(1, B_LOC):
                sts[b] = emit_A(b)
                emit_B(b - 1, sts.pop(b - 1))
                emit_A2(b, sts[b])
            emit_B(B_LOC - 1, sts.pop(B_LOC - 1))

    nc.compile()
    return nc


_CACHE: dict = {}
tap_offsets: list = []


def _prepare(x, conv_w0, conv_w1, conv_w2, proj_w, proj_b, ln_g, ln_b):
    global tap_offsets
    x = np.asarray(x, dtype=np.float32)
    conv_w0 = np.asarray(conv_w0, np.float32)
    conv_w1 = np.asarray(conv_w1, np.float32)
    conv_w2 = np.asarray(conv_w2, np.float32)
    proj_w = np.asarray(proj_w, np.float32)
    proj_b = np.asarray(proj_b, np.float32)
    ln_g = np.asarray(ln_g, np.float32)
    ln_b = np.asarray(ln_b, np.float32)

    offs, tab = _tap_table(conv_w0, conv_w1, conv_w2)
    # order taps so the full-range center tap (offset 0) initializes acc
    order = sorted(range(len(offs)), key=lambda j: (offs[j] != 0, j))
    tap_offsets = [offs[j] for j in order]
    tab = tab[:, order]

    need_pb = bool(np.any(proj_b != 0.0))
    need_gb = bool(np.any(ln_g != 1.0) or np.any(ln_b != 0.0))
    key = (len(tap_offsets), need_pb, need_gb)

    lags = _cand_lags()
    tstart = (L - lags.astype(np.int64)).astype(np.int32).reshape(1, MAXC)
    invcnt = (1.0 / ((L - lags.astype(np.float64)) * D * D)).astype(
        np.float32).reshape(MAXC, 1)

    xbf = x.astype(ml_dtypes.bfloat16)
    idt = np.eye(P, dtype=ml_dtypes.bfloat16)
    pwt = np.ascontiguousarray(proj_w.T).astype(ml_dtypes.bfloat16)
    ktap = tab.astype(np.float32)

    in_maps = []
    for c in range(N_CORES):
        m = {
            "x": np.ascontiguousarray(x[c * B_LOC:(c + 1) * B_LOC]),
            "xbf": np.ascontiguousarray(xbf[c * B_LOC:(c + 1) * B_LOC]),
            "pwt": pwt,
            "ktap": ktap,
            "tstart": tstart,
            "invcnt": invcnt,
            "idt": idt,
        }
        if need_pb:
            m["pb"] = proj_b.reshape(1, D)
        if need_gb:
            m["lng"] = ln_g.reshape(1, D)
            m["lnb"] = ln_b.reshape(1, D)
        in_maps.append(m)
    return key, in_maps


def kernel(x, conv_w0, conv_w1, conv_w2, proj_w, proj_b, ln_g, ln_b):
    key, in_maps = _prepare(
        x, conv_w0, conv_w1, conv_w2, proj_w, proj_b, ln_g, ln_b
    )
    if key not in _CACHE:
        _CACHE[key] = _build(*key)
    nc = _CACHE[key]
    res = run_bass_kernel_spmd(nc, in_maps, list(range(N_CORES)))
    out = np.concatenate([res.results[c]["out"] for c in range(N_CORES)], axis=0)
    return out.astype(np.float32)
